# revision 37
# baseline (speedup 1.0000x reference)
"""AdaptiveTopologyLayer Trainium2 kernel (8 NeuronCores, batch-parallel).

Math (reference):
    adj  = sigmoid(adj_param); thr = sort(adj)[int(0.8*N*N)]; adj = adj*(adj>thr)*(1-I)
    h    = relu(einsum('bnd,nde->bne', x, W1) + b1)
    node = einsum('bnd,nde->bne', h, W2) + b2
    mixed= einsum('ij,bjd->bid', adj, node)
    g    = relu((0.5*mixed + 0.5*node).reshape(B,N*D) @ gpW1 + gpb1)
    out  = g @ gpW2 + gpb2                  -> returns (out, adj)

Algebraic folds (batch independent, done once on host): everything between
the two relus is LINEAR, so it collapses into a single effective weight.
  1. The adjacency mix only feeds the linear gpW1 layer:
        gpW1eff[(j,d),e] = 0.5*gpW1[(j,d),e] + 0.5*sum_i adj[i,j]*gpW1[(i,d),e]
  2. W2 folds through:  effW2[j] = W2[j] @ gpW1eff[j]   ([64,64]@[64,64]/node)
  3. b2 folds into the bias: gpb1eff[e] = gpb1[e] + sum gpW1eff[(j,d),e]*b2[j,d]
Batch path becomes: h = relu(W1.T x + b1);  g = relu(effW2.T h + gpb1eff);
out = gpW2.T g + gpb2.  This removes the B*N*N*D adjacency einsum and the
stage-2 grouped GEMM from the per-batch work.

Sharding: data-parallel over batch (256 rows/core), weights replicated, no
cross-core communication. Device layout: features on SBUF partitions, batch
on the free dim (N=256 moving operand). Per 128-feature chunk (a node pair):
  MM1 (block-diag W1 pair, K=128, bf16) -> PSUM[128,256]
  relu+b1 -> bf16 h   (whole-chunk op, ALTERNATING between ACT and DVE by
                       chunk parity: halves each engine's instruction count
                       and gives the stage-4 matmul a single producer)
  MM4 (effW2 chunk, M=64)               -> accumulating PSUM[64,256]
The chunk loop is software-pipelined in groups of 6 (7 PSUM buffers) so the
TensorEngine's in-order stream never waits on the relu and semaphore waits
are elided for all but the first matmul of each group.
"""

import os
import numpy as np

GRID = 16
N = GRID * GRID          # 256 nodes
D = 64                   # embed dim
ND = N * D               # 16384
B = 2048                 # batch
SPARSITY = 0.8
NCORES = 8
BL = B // NCORES         # 256 batch rows per core
NCHUNK = ND // 128       # 128 chunks = node pairs
ACOL = 112               # relu columns on ScalarE (rest on VectorE)

_STATE = {}
LAST_EXEC_NS = None
LAST_RESULTS = None


def _build_bass():
    import concourse.bacc as bacc
    import concourse.tile as tile
    from concourse import mybir

    f32 = mybir.dt.float32
    bf16 = mybir.dt.bfloat16
    AF = mybir.ActivationFunctionType
    Alu = mybir.AluOpType

    nc = bacc.Bacc()

    xT = nc.dram_tensor("xT", [(NCHUNK // 16) * 128, 16 * BL], bf16, kind="ExternalInput")
    w1p = nc.dram_tensor("w1p", [128, 128 * NCHUNK], bf16, kind="ExternalInput")
    effp = nc.dram_tensor("effp", [128, 64 * NCHUNK], bf16, kind="ExternalInput")
    b1t = nc.dram_tensor("b1t", [128, NCHUNK], f32, kind="ExternalInput")
    gpb1 = nc.dram_tensor("gpb1", [64, 1], f32, kind="ExternalInput")
    gpw2 = nc.dram_tensor("gpw2", [64, 32], bf16, kind="ExternalInput")
    gpb2 = nc.dram_tensor("gpb2", [32, 1], f32, kind="ExternalInput")
    outd = nc.dram_tensor("out", [32, BL], f32, kind="ExternalOutput")

    XG = 16                       # chunks per x DMA group (1 MiB bf16)
    NG = NCHUNK // XG
    WSPLIT = 4

    with tile.TileContext(nc) as tc:
        with (
            tc.tile_pool(name="consts", bufs=1) as consts,
            tc.tile_pool(name="xg", bufs=3) as xgp,
            tc.tile_pool(name="act", bufs=14) as actp,
            tc.tile_pool(name="ph", bufs=7, space="PSUM") as php,
            tc.tile_pool(name="pacc", bufs=1, space="PSUM") as paccp,
        ):
            b1sb = consts.tile([128, NCHUNK], f32)
            gpb1sb = consts.tile([64, 1], f32)
            gpw2sb = consts.tile([64, 32], bf16)
            gpb2sb = consts.tile([32, 1], f32)
            nc.gpsimd.dma_start(out=b1sb[:], in_=b1t[:])
            nc.gpsimd.dma_start(out=gpb1sb[:], in_=gpb1[:])
            nc.gpsimd.dma_start(out=gpw2sb[:], in_=gpw2[:])
            nc.gpsimd.dma_start(out=gpb2sb[:], in_=gpb2[:])

            # big weights on the scalar HWDGE ring, interleaved by first use
            w1sb = consts.tile([128, 128 * NCHUNK], bf16)
            effsb = consts.tile([128, 64 * NCHUNK], bf16)
            # weights stream per 16-chunk group, prefetched 2 groups ahead of
            # use so they share HBM bandwidth smoothly with the x stream
            def weight_group_dma(g):
                c0, c1 = XG * g, XG * (g + 1)
                nc.scalar.dma_start(
                    out=w1sb[:, 128 * c0 : 128 * c1], in_=w1p[:, 128 * c0 : 128 * c1]
                )
                nc.scalar.dma_start(
                    out=effsb[:, 64 * c0 : 64 * c1], in_=effp[:, 64 * c0 : 64 * c1]
                )

            nc.scalar.dma_start(out=w1sb[:, 0:256], in_=w1p[:, 0:256])
            nc.scalar.dma_start(out=w1sb[:, 256 : 128 * XG], in_=w1p[:, 256 : 128 * XG])
            nc.scalar.dma_start(out=effsb[:, 0 : 64 * XG], in_=effp[:, 0 : 64 * XG])
            weight_group_dma(1)

            pg = paccp.tile([64, BL], f32)

            # host pre-arranged: row (g*128+p) holds chunk-major batch cols
            xt_g = xT[:].rearrange("(g p) (c b) -> g p c b", g=NG, p=128, c=XG)

            # software pipeline: stage A(c) = MM1 + relu; stage B(c) = MM4.
            state = {"xg": None}
            hs = [None] * NCHUNK

            def stage_a(c):
                g, cl = divmod(c, XG)
                if cl == 0:
                    if 2 <= g + 2 < NG:
                        weight_group_dma(g + 2)
                    state["xg"] = xgp.tile([128, XG, BL], bf16, tag="xg", name=f"xg{g}")
                    if g == 0:
                        nc.sync.dma_start(
                            out=state["xg"][:, 0:2, :], in_=xt_g[g][:, 0:2, :]
                        )
                        nc.sync.dma_start(
                            out=state["xg"][:, 2:, :], in_=xt_g[g][:, 2:, :]
                        )
                    else:
                        nc.sync.dma_start(out=state["xg"][:], in_=xt_g[g])
                xc = state["xg"][:, cl, :]
                ph = php.tile([128, BL], f32, tag="ph", name=f"ph{c}")
                nc.tensor.matmul(ph[:], lhsT=w1sb[:, 128 * c : 128 * (c + 1)], rhs=xc)
                hsb = actp.tile([128, BL], bf16, tag="h", name=f"h{c}")
                bias = b1sb[:, c : c + 1]
                if c % 2 == 0:
                    nc.scalar.activation(hsb[:], ph[:], AF.Relu, bias=bias)
                else:
                    nc.vector.tensor_scalar(
                        hsb[:], ph[:], bias, 0.0, Alu.add, Alu.max
                    )
                hs[c] = hsb

            def stage_b(c):
                nc.tensor.matmul(
                    pg[:],
                    lhsT=effsb[:, 64 * c : 64 * (c + 1)],
                    rhs=hs[c][:],
                    start=(c == 0),
                    stop=(c == NCHUNK - 1),
                )
                hs[c] = None

            GRP = 6
            done_a = 0
            done_b = 0
            while done_b < NCHUNK:
                hi = min(done_a + GRP, NCHUNK)
                for c in range(done_a, hi):
                    stage_a(c)
                done_a = hi
                for c in range(done_b, min(done_b + GRP, done_a - GRP if done_a < NCHUNK else NCHUNK)):
                    stage_b(c)
                    done_b = c + 1

            # epilogue: g = relu(g_pre + gpb1eff); out = gpW2.T @ g + gpb2
            gsb = consts.tile([64, BL], bf16)
            nc.scalar.activation(gsb[:], pg[:], AF.Relu, bias=gpb1sb[:])
            po = php.tile([32, BL], f32, tag="ph", name="po")
            nc.tensor.matmul(po[:], lhsT=gpw2sb[:], rhs=gsb[:])
            osb = consts.tile([32, BL], f32)
            nc.vector.tensor_scalar_add(osb[:], po[:], gpb2sb[:])
            nc.sync.dma_start(out=outd[:], in_=osb[:])

    nc.finalize()
    return nc


def _block_diag_pairs(w, bf):
    """[256,64,64] (n,d,e) -> [128, NCHUNK*128] block-diag per node pair."""
    out = np.zeros((128, NCHUNK, 128), dtype=np.float32)
    out[0:64, :, 0:64] = w[0::2].transpose(1, 0, 2)
    out[64:128, :, 64:128] = w[1::2].transpose(1, 0, 2)
    return np.ascontiguousarray(out.reshape(128, NCHUNK * 128)).astype(bf)


def _pack_pairs(w):
    """[256,64,64] (n,d,e) -> [128, NCHUNK*64] with [nl*64+d, c*64+e] = w[2c+nl,d,e]."""
    return np.ascontiguousarray(
        w.reshape(NCHUNK, 2, 64, 64).transpose(1, 2, 0, 3).reshape(128, NCHUNK * 64)
    )


def _host_adj(adj_param):
    s = 1.0 / (1.0 + np.exp(-adj_param.astype(np.float64)))
    adj = s.astype(np.float32)
    flat = np.sort(adj.reshape(-1))
    thr = flat[int(SPARSITY * N * N)]
    adj = adj * (adj > thr)
    adj *= 1.0 - np.eye(N, dtype=np.float32)
    return adj


def kernel(x, adj_param, W1, b1, W2, b2, gpW1, gpb1, gpW2, gpb2):
    global LAST_EXEC_NS, LAST_RESULTS
    import ml_dtypes
    from concourse.bass_utils import run_bass_kernel_spmd

    bf = ml_dtypes.bfloat16
    x = np.asarray(x, dtype=np.float32)
    adj_param = np.asarray(adj_param, dtype=np.float32)
    W1 = np.asarray(W1, dtype=np.float32)
    b1 = np.asarray(b1, dtype=np.float32)
    W2 = np.asarray(W2, dtype=np.float32)
    b2 = np.asarray(b2, dtype=np.float32)
    gpW1 = np.asarray(gpW1, dtype=np.float32)
    gpb1v = np.asarray(gpb1, dtype=np.float32)
    gpW2 = np.asarray(gpW2, dtype=np.float32)
    gpb2v = np.asarray(gpb2, dtype=np.float32)

    adj = _host_adj(adj_param)

    # fold 1: adjacency mix -> effective global-pool weight
    gw = gpW1.reshape(N, D, D)
    mixw = np.tensordot(adj, gw, axes=([0], [0]))     # [j,d,e] = sum_i adj[i,j]*gw[i,d,e]
    eff = (0.5 * gpW1 + 0.5 * mixw.reshape(ND, D)).astype(np.float32)
    # fold 2: b2 through eff -> bias of the global pool
    gpb1eff = gpb1v.astype(np.float64) + eff.astype(np.float64).T @ b2.reshape(ND).astype(
        np.float64
    )
    gpb1eff = gpb1eff.astype(np.float32)
    # fold 3: W2 through eff (everything between the relus is linear)
    effW2 = np.einsum(
        "jde,jef->jdf", W2.astype(np.float64), eff.reshape(N, D, D).astype(np.float64)
    ).astype(np.float32)

    in_common = {
        "w1p": _block_diag_pairs(W1, bf),
        "effp": _pack_pairs(effW2).astype(bf),
        "b1t": np.ascontiguousarray(
            b1.reshape(NCHUNK, 2, 64).transpose(1, 2, 0).reshape(128, NCHUNK)
        ),
        "gpb1": np.ascontiguousarray(gpb1eff.reshape(64, 1)),
        "gpw2": np.ascontiguousarray(gpW2).astype(bf),
        "gpb2": np.ascontiguousarray(gpb2v.reshape(32, 1)),
    }
    in_maps = []
    for core in range(NCORES):
        xs = np.ascontiguousarray(x[core * BL : (core + 1) * BL, :].T).astype(bf)
        # group-major layout: [g*128+p, c*BL+b] so each group DMA is
        # contiguous per partition (few large descriptors)
        NG, XG = NCHUNK // 16, 16
        xs2 = np.ascontiguousarray(
            xs.reshape(NG, XG, 128, BL).transpose(0, 2, 1, 3).reshape(NG * 128, XG * BL)
        )
        in_maps.append({**in_common, "xT": xs2})

    if "nc" not in _STATE:
        _STATE["nc"] = _build_bass()
    nc = _STATE["nc"]

    trace = bool(os.environ.get("BASS_KERNEL_TRACE"))
    res = run_bass_kernel_spmd(nc, in_maps, core_ids=list(range(NCORES)), trace=trace)
    LAST_RESULTS = res
    LAST_EXEC_NS = res.exec_time_ns

    out = np.concatenate([r["out"].T for r in res.results], axis=0)
    return out.astype(np.float32), adj


# revision 38
# speedup vs baseline: 1.0288x; 1.0288x over previous
"""AdaptiveTopologyLayer Trainium2 kernel (8 NeuronCores, batch-parallel).

Math (reference):
    adj  = sigmoid(adj_param); thr = sort(adj)[int(0.8*N*N)]; adj = adj*(adj>thr)*(1-I)
    h    = relu(einsum('bnd,nde->bne', x, W1) + b1)
    node = einsum('bnd,nde->bne', h, W2) + b2
    mixed= einsum('ij,bjd->bid', adj, node)
    g    = relu((0.5*mixed + 0.5*node).reshape(B,N*D) @ gpW1 + gpb1)
    out  = g @ gpW2 + gpb2                  -> returns (out, adj)

Algebraic folds (batch independent, done once on host): everything between
the two relus is LINEAR, so it collapses into a single effective weight.
  1. The adjacency mix only feeds the linear gpW1 layer:
        gpW1eff[(j,d),e] = 0.5*gpW1[(j,d),e] + 0.5*sum_i adj[i,j]*gpW1[(i,d),e]
  2. W2 folds through:  effW2[j] = W2[j] @ gpW1eff[j]   ([64,64]@[64,64]/node)
  3. b2 folds into the bias: gpb1eff[e] = gpb1[e] + sum gpW1eff[(j,d),e]*b2[j,d]
Batch path becomes: h = relu(W1.T x + b1);  g = relu(effW2.T h + gpb1eff);
out = gpW2.T g + gpb2.  This removes the B*N*N*D adjacency einsum and the
stage-2 grouped GEMM from the per-batch work.

Sharding: data-parallel over batch (256 rows/core), weights replicated, no
cross-core communication. Device layout: features on SBUF partitions, batch
on the free dim (N=256 moving operand). Per 128-feature chunk (a node pair):
  MM1 (block-diag W1 pair, K=128, bf16) -> PSUM[128,256]
  relu+b1 -> bf16 h   (whole-chunk op, ALTERNATING between ACT and DVE by
                       chunk parity: halves each engine's instruction count
                       and gives the stage-4 matmul a single producer)
  MM4 (effW2 chunk, M=64)               -> accumulating PSUM[64,256]
The chunk loop is software-pipelined in groups of 6 (7 PSUM buffers) so the
TensorEngine's in-order stream never waits on the relu and semaphore waits
are elided for all but the first matmul of each group.
"""

import os
import numpy as np

GRID = 16
N = GRID * GRID          # 256 nodes
D = 64                   # embed dim
ND = N * D               # 16384
B = 2048                 # batch
SPARSITY = 0.8
NCORES = 8
BL = B // NCORES         # 256 batch rows per core
NCHUNK = ND // 128       # 128 chunks = node pairs
ACOL = 112               # relu columns on ScalarE (rest on VectorE)

_STATE = {}
LAST_EXEC_NS = None
LAST_RESULTS = None


def _build_bass():
    import concourse.bacc as bacc
    import concourse.tile as tile
    from concourse import mybir

    f32 = mybir.dt.float32
    bf16 = mybir.dt.bfloat16
    AF = mybir.ActivationFunctionType
    Alu = mybir.AluOpType

    nc = bacc.Bacc()

    xT = nc.dram_tensor("xT", [(NCHUNK // 16) * 128, 16 * BL], bf16, kind="ExternalInput")
    w1p = nc.dram_tensor("w1p", [128, 128 * NCHUNK], bf16, kind="ExternalInput")
    effp = nc.dram_tensor("effp", [128, 64 * NCHUNK], bf16, kind="ExternalInput")
    b1t = nc.dram_tensor("b1t", [128, NCHUNK], f32, kind="ExternalInput")
    gpb1 = nc.dram_tensor("gpb1", [64, 1], f32, kind="ExternalInput")
    gpw2 = nc.dram_tensor("gpw2", [64, 32], bf16, kind="ExternalInput")
    gpb2 = nc.dram_tensor("gpb2", [32, 1], f32, kind="ExternalInput")
    outd = nc.dram_tensor("out", [32, BL], f32, kind="ExternalOutput")

    XG = 16                       # chunks per x DMA group (1 MiB bf16)
    NG = NCHUNK // XG
    WSPLIT = 4

    with tile.TileContext(nc) as tc:
        with (
            tc.tile_pool(name="consts", bufs=1) as consts,
            tc.tile_pool(name="xg", bufs=3) as xgp,
            tc.tile_pool(name="act", bufs=14) as actp,
            tc.tile_pool(name="ph", bufs=7, space="PSUM") as php,
            tc.tile_pool(name="pacc", bufs=1, space="PSUM") as paccp,
        ):
            b1sb = consts.tile([128, NCHUNK], f32)
            gpb1sb = consts.tile([64, 1], f32)
            gpw2sb = consts.tile([64, 32], bf16)
            gpb2sb = consts.tile([32, 1], f32)
            nc.gpsimd.dma_start(out=b1sb[:], in_=b1t[:])
            nc.gpsimd.dma_start(out=gpb1sb[:], in_=gpb1[:])
            nc.gpsimd.dma_start(out=gpw2sb[:], in_=gpw2[:])
            nc.gpsimd.dma_start(out=gpb2sb[:], in_=gpb2[:])

            # big weights on the scalar HWDGE ring, interleaved by first use
            w1sb = consts.tile([128, 128 * NCHUNK], bf16)
            effsb = consts.tile([128, 64 * NCHUNK], bf16)
            # weights stream per 16-chunk group, prefetched 2 groups ahead of
            # use so they share HBM bandwidth smoothly with the x stream
            def weight_group_dma(g):
                c0, c1 = XG * g, XG * (g + 1)
                nc.scalar.dma_start(
                    out=w1sb[:, 128 * c0 : 128 * c1], in_=w1p[:, 128 * c0 : 128 * c1]
                )
                nc.scalar.dma_start(
                    out=effsb[:, 64 * c0 : 64 * c1], in_=effp[:, 64 * c0 : 64 * c1]
                )

            nc.scalar.dma_start(out=w1sb[:, 0:256], in_=w1p[:, 0:256])
            nc.scalar.dma_start(out=w1sb[:, 256 : 128 * XG], in_=w1p[:, 256 : 128 * XG])
            nc.scalar.dma_start(out=effsb[:, 0 : 64 * XG], in_=effp[:, 0 : 64 * XG])
            weight_group_dma(1)

            pg = paccp.tile([64, BL], f32)

            # host pre-arranged: row (g*128+p) holds chunk-major batch cols
            xt_g = xT[:].rearrange("(g p) (c b) -> g p c b", g=NG, p=128, c=XG)

            # software pipeline: stage A(c) = MM1 + relu; stage B(c) = MM4.
            state = {"xg": None}
            hs = [None] * NCHUNK

            def stage_a(c):
                g, cl = divmod(c, XG)
                if cl == 0:
                    if 2 <= g + 2 < NG:
                        weight_group_dma(g + 2)
                    state["xg"] = xgp.tile([128, XG, BL], bf16, tag="xg", name=f"xg{g}")
                    if g == 0:
                        nc.sync.dma_start(
                            out=state["xg"][:, 0:2, :], in_=xt_g[g][:, 0:2, :]
                        )
                        nc.sync.dma_start(
                            out=state["xg"][:, 2:, :], in_=xt_g[g][:, 2:, :]
                        )
                    else:
                        nc.sync.dma_start(out=state["xg"][:], in_=xt_g[g])
                xc = state["xg"][:, cl, :]
                ph = php.tile([128, BL], f32, tag="ph", name=f"ph{c}")
                nc.tensor.matmul(ph[:], lhsT=w1sb[:, 128 * c : 128 * (c + 1)], rhs=xc)
                hsb = actp.tile([128, BL], bf16, tag="h", name=f"h{c}")
                bias = b1sb[:, c : c + 1]
                if c % 2 == 0:
                    nc.scalar.activation(hsb[:], ph[:], AF.Relu, bias=bias)
                else:
                    nc.vector.tensor_scalar(
                        hsb[:], ph[:], bias, 0.0, Alu.add, Alu.max
                    )
                hs[c] = hsb

            def stage_b(c):
                nc.tensor.matmul(
                    pg[:],
                    lhsT=effsb[:, 64 * c : 64 * (c + 1)],
                    rhs=hs[c][:],
                    start=(c == 0),
                    stop=(c == NCHUNK - 1),
                )
                hs[c] = None

            GRP = 4
            done_a = 0
            done_b = 0
            while done_b < NCHUNK:
                hi = min(done_a + GRP, NCHUNK)
                for c in range(done_a, hi):
                    stage_a(c)
                done_a = hi
                for c in range(done_b, min(done_b + GRP, done_a - GRP if done_a < NCHUNK else NCHUNK)):
                    stage_b(c)
                    done_b = c + 1

            # epilogue: g = relu(g_pre + gpb1eff); out = gpW2.T @ g + gpb2
            gsb = consts.tile([64, BL], bf16)
            nc.scalar.activation(gsb[:], pg[:], AF.Relu, bias=gpb1sb[:])
            po = php.tile([32, BL], f32, tag="ph", name="po")
            nc.tensor.matmul(po[:], lhsT=gpw2sb[:], rhs=gsb[:])
            osb = consts.tile([32, BL], f32)
            nc.vector.tensor_scalar_add(osb[:], po[:], gpb2sb[:])
            nc.sync.dma_start(out=outd[:], in_=osb[:])

    nc.finalize()
    return nc


def _block_diag_pairs(w, bf):
    """[256,64,64] (n,d,e) -> [128, NCHUNK*128] block-diag per node pair."""
    out = np.zeros((128, NCHUNK, 128), dtype=np.float32)
    out[0:64, :, 0:64] = w[0::2].transpose(1, 0, 2)
    out[64:128, :, 64:128] = w[1::2].transpose(1, 0, 2)
    return np.ascontiguousarray(out.reshape(128, NCHUNK * 128)).astype(bf)


def _pack_pairs(w):
    """[256,64,64] (n,d,e) -> [128, NCHUNK*64] with [nl*64+d, c*64+e] = w[2c+nl,d,e]."""
    return np.ascontiguousarray(
        w.reshape(NCHUNK, 2, 64, 64).transpose(1, 2, 0, 3).reshape(128, NCHUNK * 64)
    )


def _host_adj(adj_param):
    s = 1.0 / (1.0 + np.exp(-adj_param.astype(np.float64)))
    adj = s.astype(np.float32)
    flat = np.sort(adj.reshape(-1))
    thr = flat[int(SPARSITY * N * N)]
    adj = adj * (adj > thr)
    adj *= 1.0 - np.eye(N, dtype=np.float32)
    return adj


def kernel(x, adj_param, W1, b1, W2, b2, gpW1, gpb1, gpW2, gpb2):
    global LAST_EXEC_NS, LAST_RESULTS
    import ml_dtypes
    from concourse.bass_utils import run_bass_kernel_spmd

    bf = ml_dtypes.bfloat16
    x = np.asarray(x, dtype=np.float32)
    adj_param = np.asarray(adj_param, dtype=np.float32)
    W1 = np.asarray(W1, dtype=np.float32)
    b1 = np.asarray(b1, dtype=np.float32)
    W2 = np.asarray(W2, dtype=np.float32)
    b2 = np.asarray(b2, dtype=np.float32)
    gpW1 = np.asarray(gpW1, dtype=np.float32)
    gpb1v = np.asarray(gpb1, dtype=np.float32)
    gpW2 = np.asarray(gpW2, dtype=np.float32)
    gpb2v = np.asarray(gpb2, dtype=np.float32)

    adj = _host_adj(adj_param)

    # fold 1: adjacency mix -> effective global-pool weight
    gw = gpW1.reshape(N, D, D)
    mixw = np.tensordot(adj, gw, axes=([0], [0]))     # [j,d,e] = sum_i adj[i,j]*gw[i,d,e]
    eff = (0.5 * gpW1 + 0.5 * mixw.reshape(ND, D)).astype(np.float32)
    # fold 2: b2 through eff -> bias of the global pool
    gpb1eff = gpb1v.astype(np.float64) + eff.astype(np.float64).T @ b2.reshape(ND).astype(
        np.float64
    )
    gpb1eff = gpb1eff.astype(np.float32)
    # fold 3: W2 through eff (everything between the relus is linear)
    effW2 = np.einsum(
        "jde,jef->jdf", W2.astype(np.float64), eff.reshape(N, D, D).astype(np.float64)
    ).astype(np.float32)

    in_common = {
        "w1p": _block_diag_pairs(W1, bf),
        "effp": _pack_pairs(effW2).astype(bf),
        "b1t": np.ascontiguousarray(
            b1.reshape(NCHUNK, 2, 64).transpose(1, 2, 0).reshape(128, NCHUNK)
        ),
        "gpb1": np.ascontiguousarray(gpb1eff.reshape(64, 1)),
        "gpw2": np.ascontiguousarray(gpW2).astype(bf),
        "gpb2": np.ascontiguousarray(gpb2v.reshape(32, 1)),
    }
    in_maps = []
    for core in range(NCORES):
        xs = np.ascontiguousarray(x[core * BL : (core + 1) * BL, :].T).astype(bf)
        # group-major layout: [g*128+p, c*BL+b] so each group DMA is
        # contiguous per partition (few large descriptors)
        NG, XG = NCHUNK // 16, 16
        xs2 = np.ascontiguousarray(
            xs.reshape(NG, XG, 128, BL).transpose(0, 2, 1, 3).reshape(NG * 128, XG * BL)
        )
        in_maps.append({**in_common, "xT": xs2})

    if "nc" not in _STATE:
        _STATE["nc"] = _build_bass()
    nc = _STATE["nc"]

    trace = bool(os.environ.get("BASS_KERNEL_TRACE"))
    res = run_bass_kernel_spmd(nc, in_maps, core_ids=list(range(NCORES)), trace=trace)
    LAST_RESULTS = res
    LAST_EXEC_NS = res.exec_time_ns

    out = np.concatenate([r["out"].T for r in res.results], axis=0)
    return out.astype(np.float32), adj
